# revision 9
# baseline (speedup 1.0000x reference)
"""Multi-head self-attention (B=2, S=2048, D=1024, H=16, causal) on 8 TRN2 cores.

Sharding: tensor-parallel over heads. Core c owns heads {2c, 2c+1}:
  - Wq/Wk/Wv column-sharded: core c gets columns [128c, 128c+128).
  - Each core computes Q^T,K^T,V^T (head-dim on partitions) for its heads,
    both batches, via f32r matmuls against x^T; V is PE-transposed back to
    seq-on-partitions layout.
  - Attention in transposed-scores layout: S^T[k, q] tiles, so softmax
    denominators come free from an extra ones-column in V (row 64 of the
    AV psum accumulates sum_k P^T[k, q]).  Causal masking is folded into
    the scores matmul accumulation as a -BIG upper-triangular bias matmul
    (bf16) on diagonal tiles; fully-masked column blocks are skipped.
  - Normalize Z^T by the per-q reciprocal (PE broadcast of the recip row).
  - AllToAll exchanges Z^T so core j holds all 1024 head-dims for sequence
    block j (512 rows); the output projection then needs full Wo rows and
    produces a disjoint output slab per core (no all-reduce).
"""

import ml_dtypes
import numpy as np

import concourse.bass as bass
import concourse.mybir as mybir
import concourse.tile as tile
from concourse import bacc
from concourse.bass_utils import run_bass_kernel_spmd

N_CORES = 8
B, S, D = 2, 2048, 1024
H = 16
HD = D // H          # 64
BS = B * S           # 4096 flattened tokens
CD = 2 * HD          # 128 head-dims per core
NBLK = BS // 512     # 8 sequence blocks of 512
NM = S // 512        # 4 q-chunks per batch
BIG = 30000.0
SCALE = 1.0 / np.sqrt(HD)

F32 = mybir.dt.float32
BF16 = mybir.dt.bfloat16
F32R = mybir.dt.float32r
EXP = mybir.ActivationFunctionType.Exp

_CACHE = {}


def build_nc(with_collective=True, reps=1):
    nc = bacc.Bacc("TRN2", target_bir_lowering=False, debug=False, num_devices=N_CORES)

    xT = nc.dram_tensor("xT", [D, BS], F32, kind="ExternalInput").ap()
    wq = nc.dram_tensor("wq", [D, CD], F32, kind="ExternalInput").ap()
    wk = nc.dram_tensor("wk", [D, CD], F32, kind="ExternalInput").ap()
    wv = nc.dram_tensor("wv", [D, CD], F32, kind="ExternalInput").ap()
    wo = nc.dram_tensor("wo", [D, D], F32, kind="ExternalInput").ap()
    bo = nc.dram_tensor("bo", [1, D], F32, kind="ExternalInput").ap()
    masku = nc.dram_tensor("masku", [128, 128], BF16, kind="ExternalInput").ap()
    ident = nc.dram_tensor("ident", [128, 128], BF16, kind="ExternalInput").ap()
    identr = nc.dram_tensor("identr", [128, 128], F32, kind="ExternalInput").ap()
    onesr = nc.dram_tensor("onesr", [128, 128], F32, kind="ExternalInput").ap()
    out = nc.dram_tensor("out", [512, D], F32, kind="ExternalOutput").ap()

    r = F32R

    with tile.TileContext(nc) as tc:
        with (
            tc.tile_pool(name="const", bufs=1) as constp,
            tc.tile_pool(name="persist", bufs=1) as persist,
            tc.tile_pool(name="xt", bufs=2) as xtp,
            tc.tile_pool(name="work", bufs=3) as work,
            tc.tile_pool(name="pp", bufs=2, space="PSUM") as pp,
            tc.tile_pool(name="psc", bufs=2, space="PSUM") as psc,
            tc.tile_pool(name="pz", bufs=2, space="PSUM") as pz,
            tc.tile_pool(name="dram", bufs=1, space="DRAM") as dram,
        ):
            # ---- small constants (loaded once) ----
            masku_sb = constp.tile([128, 128], BF16)
            ident_sb = constp.tile([128, 128], BF16)
            identr_sb = constp.tile([128, 128], r)
            ones_sb = constp.tile([128, 128], r)
            nc.sync.dma_start(masku_sb[:], masku)
            nc.sync.dma_start(ident_sb[:], ident)
            nc.sync.dma_start(identr_sb[:], identr.bitcast(r))
            nc.sync.dma_start(ones_sb[:], onesr.bitcast(r))
            cc_in = dram.tile([NBLK, 128, 512], F32)
            cc_out = dram.tile([NBLK, 128, 512], F32)
            xTr = xT.rearrange("(e p) s -> p e s", p=128).bitcast(r)

            for _rep in range(reps):
                _body(nc, constp, persist, xtp, work, pp, psc, pz,
                      xTr, wq, wk, wv, wo, bo, out,
                      masku_sb, ident_sb, identr_sb, ones_sb, cc_in, cc_out,
                      with_collective)

    nc.compile()
    return nc


def _body(nc, constp, persist, xtp, work, pp, psc, pz,
          xTr, wq, wk, wv, wo, bo, out,
          masku_sb, ident_sb, identr_sb, ones_sb, cc_in, cc_out,
          with_collective):
    r = F32R

    # ---- projection weights ----
    wq_sb = constp.tile([128, 8, CD], r, tag="wq", name="wq_sb")
    wk_sb = constp.tile([128, 8, CD], r, tag="wk", name="wk_sb")
    wv_sb = constp.tile([128, 8, CD], r, tag="wv", name="wv_sb")
    nc.sync.dma_start(wq_sb[:], wq.rearrange("(e p) c -> p e c", p=128).bitcast(r))
    nc.sync.dma_start(wk_sb[:], wk.rearrange("(e p) c -> p e c", p=128).bitcast(r))
    nc.sync.dma_start(wv_sb[:], wv.rearrange("(e p) c -> p e c", p=128).bitcast(r))

    # ---- persistent activations ----
    qt_sb = persist.tile([128, BS], r, tag="qt", name="qt_sb")
    kt_sb = persist.tile([128, BS], r, tag="kt", name="kt_sb")
    vt_sb = persist.tile([128, BS], r, tag="vt", name="vt_sb")
    v_sb = persist.tile([128, 32, 130], r, tag="v", name="v_sb")
    nc.vector.tensor_copy(v_sb[:, :, 64], ones_sb[:, 0:32])
    nc.vector.tensor_copy(v_sb[:, :, 129], ones_sb[:, 0:32])

    def proj_chunk(sc):
        sl = bass.ts(sc, 512)
        xt_t = xtp.tile([128, 8, 512], r, tag="xt", name=f"xt{sc}")
        for e in range(8):
            nc.sync.dma_start(xt_t[:, e, :], xTr[:, e, sl])
        for w_sb, o_sb in ((wq_sb, qt_sb), (wk_sb, kt_sb), (wv_sb, vt_sb)):
            p_ps = pp.tile([128, 512], F32, tag="p", name=f"pp{sc}")
            for e in range(8):
                nc.tensor.matmul(
                    p_ps[:], w_sb[:, e, :], xt_t[:, e, :],
                    start=(e == 0), stop=(e == 7),
                )
            nc.vector.tensor_copy(o_sb[:, sl], p_ps[:])
        for st in range(4):
            tt = 4 * sc + st
            v_ps = pp.tile([128, 128], r, tag="p", name=f"vtp{sc}{st}")
            nc.tensor.transpose(
                v_ps[:], vt_sb[:, 512 * sc + 128 * st:512 * sc + 128 * st + 128],
                identr_sb[:],
            )
            nc.vector.tensor_copy(v_sb[:, tt, 0:64], v_ps[:, 0:64])
            nc.vector.tensor_copy(v_sb[:, tt, 65:129], v_ps[:, 64:128])

    def attn_chunk(b, m):
        q0 = 2048 * b + 512 * m
        z_ps = [pz.tile([65, 512], F32, tag="z", name=f"z{b}{m}{h}") for h in (0, 1)]
        last_t = 4 * m + 3
        for t in range(last_t + 1):
            k0 = 2048 * b + 128 * t
            jo = max(0, 128 * (t - 4 * m))
            s_ps = psc.tile([128, 1024], F32, tag="s", name=f"s{b}{m}{t}")
            for h in (0, 1):
                hsl = slice(64 * h, 64 * h + 64)
                nc.tensor.matmul(
                    s_ps[:, 512 * h + jo:512 * h + 512],
                    kt_sb[hsl, k0:k0 + 128],
                    qt_sb[hsl, q0 + jo:q0 + 512],
                    start=True, stop=(t < 4 * m),
                )
                if t >= 4 * m:
                    nc.tensor.matmul(
                        s_ps[:, 512 * h + jo:512 * h + jo + 128],
                        masku_sb[:], ident_sb[:],
                        start=False, stop=True,
                    )
            pt_sb = work.tile([128, 1024], r, tag="pt", name=f"pt{b}{m}{t}")
            nc.scalar.activation(
                pt_sb[:].rearrange("p (h w) -> p h w", h=2)[:, :, jo:512],
                s_ps[:].rearrange("p (h w) -> p h w", h=2)[:, :, jo:512],
                EXP, scale=float(SCALE),
            )
            for h in (0, 1):
                nc.tensor.matmul(
                    z_ps[h][:, jo:512],
                    v_sb[:, 16 * b + t, 65 * h:65 * h + 65],
                    pt_sb[:, 512 * h + jo:512 * h + 512],
                    start=(t == 0), stop=(t == last_t),
                )
        # normalize and stage for all-to-all; copy psum out early to release
        # the z bank, then finish from SBUF
        for h in (0, 1):
            zcp = work.tile([65, 512], F32, tag="zc", name=f"zc{b}{m}{h}")
            nc.vector.tensor_copy(zcp[0:64, :], z_ps[h][0:64, :])
            recip = work.tile([65, 512], r, tag="rc", name=f"rc{b}{m}{h}")
            with nc.allow_low_precision(reason="f32r is bitwise f32 here"):
                nc.vector.reciprocal(recip[64:65, :], z_ps[h][64:65, :])
            bc_ps = psc.tile([64, 512], F32, tag="s", name=f"bc{b}{m}{h}")
            nc.tensor.matmul(
                bc_ps[:], ones_sb[64:65, 0:64], recip[64:65, :],
                start=True, stop=True,
            )
            bc_sb = work.tile([64, 512], F32, tag="bc", name=f"bcs{b}{m}{h}")
            nc.vector.tensor_copy(bc_sb[:], bc_ps[:])
            zt_sb = work.tile([64, 512], F32, tag="zt", name=f"zt{b}{m}{h}")
            nc.vector.tensor_mul(zt_sb[:], zcp[0:64, :], bc_sb[:])
            j = 4 * b + m
            nc.sync.dma_start(cc_in[j, 64 * h:64 * h + 64, :], zt_sb[:])

    # ---- issue order: proj b0, then b0 attention woven with proj b1 ----
    wo_sb = persist.tile([128, 8, D], r, tag="wo", name="wo_sb")
    bo_sb = constp.tile([1, D], r, tag="bo", name="bo_sb")
    for sc in range(4):
        proj_chunk(sc)
    for m in range(NM):
        attn_chunk(0, m)
        proj_chunk(4 + m)
    nc.sync.dma_start(wo_sb[:], wo.rearrange("(i p) e -> p i e", p=128).bitcast(r))
    nc.sync.dma_start(bo_sb[:], bo.bitcast(r))
    for m in range(NM):
        attn_chunk(1, m)

    # ---- exchange Z^T: core j receives all head-dims for seq block j ----
    if with_collective:
        nc.gpsimd.collective_compute(
            "AllToAll",
            mybir.AluOpType.bypass,
            replica_groups=[list(range(N_CORES))],
            ins=[cc_in.opt()],
            outs=[cc_out.opt()],
        )
    else:
        nc.sync.dma_start(cc_out[:], cc_in[:])

    # ---- output projection: O[s_blk, :] = Z^T.T @ Wo + bo ----
    zt2_sb = persist.tile([128, 8, 512], r, tag="zt2", name="zt2_sb")
    nc.sync.dma_start(zt2_sb[:], cc_out.rearrange("i p s -> p i s").bitcast(r))

    for st in range(4):
        for e in range(2):
            o_ps = psc.tile([128, 512], F32, tag="s", name=f"o{st}{e}")
            for i in range(8):
                nc.tensor.matmul(
                    o_ps[:],
                    zt2_sb[:, i, bass.ts(st, 128)],
                    wo_sb[:, i, bass.ts(e, 512)],
                    start=(i == 0), stop=False,
                )
            nc.tensor.matmul(
                o_ps[:], ones_sb[0:1, 0:128], bo_sb[0:1, bass.ts(e, 512)],
                start=False, stop=True,
            )
            o_sb = work.tile([128, 512], F32, tag="o", name=f"os{st}{e}")
            nc.vector.tensor_copy(o_sb[:], o_ps[:])
            nc.sync.dma_start(out[bass.ts(st, 128), bass.ts(e, 512)], o_sb[:])


def _prep_inputs(inputs, Wq, Wk, Wv, Wo, bo):
    x = np.asarray(inputs, dtype=np.float32).reshape(BS, D)
    xT = np.ascontiguousarray(x.T)
    Wq = np.asarray(Wq, dtype=np.float32)
    Wk = np.asarray(Wk, dtype=np.float32)
    Wv = np.asarray(Wv, dtype=np.float32)
    Wo = np.ascontiguousarray(np.asarray(Wo, dtype=np.float32))
    bo = np.asarray(bo, dtype=np.float32).reshape(1, D)
    masku = np.triu(np.full((128, 128), -BIG, dtype=np.float32), k=1).astype(ml_dtypes.bfloat16)
    ident = np.eye(128, dtype=np.float32).astype(ml_dtypes.bfloat16)
    identr = np.eye(128, dtype=np.float32)
    onesr = np.ones((128, 128), dtype=np.float32)
    in_maps = []
    for c in range(N_CORES):
        csl = slice(CD * c, CD * (c + 1))
        in_maps.append({
            "xT": xT,
            "wq": np.ascontiguousarray(Wq[:, csl]),
            "wk": np.ascontiguousarray(Wk[:, csl]),
            "wv": np.ascontiguousarray(Wv[:, csl]),
            "wo": Wo,
            "bo": bo,
            "masku": masku,
            "ident": ident,
            "identr": identr,
            "onesr": onesr,
        })
    return in_maps


def kernel(inputs, Wq, Wk, Wv, Wo, bo):
    if "nc" not in _CACHE:
        _CACHE["nc"] = build_nc()
    nc = _CACHE["nc"]
    in_maps = _prep_inputs(inputs, Wq, Wk, Wv, Wo, bo)
    res = run_bass_kernel_spmd(nc, in_maps, core_ids=list(range(N_CORES)))
    slabs = [res.results[c]["out"] for c in range(N_CORES)]
    return np.concatenate(slabs, axis=0).reshape(B, S, D)


# revision 25
# speedup vs baseline: 36.8500x; 36.8500x over previous
"""Multi-head self-attention (B=2, S=2048, D=1024, H=16, causal) on 8 TRN2 cores.

Sharding: tensor-parallel over heads. Core c owns heads {2c, 2c+1}:
  - Wq/Wk/Wv column-sharded: core c gets columns [128c, 128c+128).
  - Each core computes Q^T,K^T,V^T (head-dim on partitions) for its heads,
    both batches, via f32r matmuls against x^T; V is PE-transposed back to
    seq-on-partitions layout.
  - Attention in transposed-scores layout: S^T[k, q] tiles, so softmax
    denominators come free from an extra ones-column in V (row 64 of the
    AV psum accumulates sum_k P^T[k, q]).  Causal masking is folded into
    the scores matmul accumulation as a -BIG upper-triangular bias matmul
    (bf16) on diagonal tiles; fully-masked column blocks are skipped.
  - Normalize Z^T by the per-q reciprocal (PE broadcast of the recip row).
  - AllToAll exchanges Z^T so core j holds all 1024 head-dims for sequence
    block j (512 rows); the output projection then needs full Wo rows and
    produces a disjoint output slab per core (no all-reduce).
"""

import ml_dtypes
import numpy as np

import concourse.bass as bass
import concourse.mybir as mybir
import concourse.tile as tile
from concourse import bacc
from concourse.bass_utils import run_bass_kernel_spmd

N_CORES = 8
B, S, D = 2, 2048, 1024
H = 16
HD = D // H          # 64
BS = B * S           # 4096 flattened tokens
CD = 2 * HD          # 128 head-dims per core
NBLK = BS // 512     # 8 sequence blocks of 512
NM = S // 512        # 4 q-chunks per batch
BIG = 30000.0
SCALE = 1.0 / np.sqrt(HD)

F32 = mybir.dt.float32
BF16 = mybir.dt.bfloat16
F32R = mybir.dt.float32r
EXP = mybir.ActivationFunctionType.Exp

_CACHE = {}


def build_nc(with_collective=True, reps=1):
    nc = bacc.Bacc("TRN2", target_bir_lowering=False, debug=False, num_devices=N_CORES)

    xT = nc.dram_tensor("xT", [D, BS], F32, kind="ExternalInput").ap()
    wq = nc.dram_tensor("wq", [D, CD], F32, kind="ExternalInput").ap()
    wk = nc.dram_tensor("wk", [D, CD], F32, kind="ExternalInput").ap()
    wv = nc.dram_tensor("wv", [D, CD], F32, kind="ExternalInput").ap()
    wo = nc.dram_tensor("wo", [D, D], F32, kind="ExternalInput").ap()
    bo = nc.dram_tensor("bo", [1, D], F32, kind="ExternalInput").ap()
    masku = nc.dram_tensor("masku", [128, 128], BF16, kind="ExternalInput").ap()
    ident = nc.dram_tensor("ident", [128, 128], BF16, kind="ExternalInput").ap()
    identr = nc.dram_tensor("identr", [128, 128], F32, kind="ExternalInput").ap()
    onesr = nc.dram_tensor("onesr", [128, 128], F32, kind="ExternalInput").ap()
    out = nc.dram_tensor("out", [512, D], F32, kind="ExternalOutput").ap()

    r = F32R

    with tile.TileContext(nc) as tc:
        with (
            tc.tile_pool(name="const", bufs=1) as constp,
            tc.tile_pool(name="persist", bufs=1) as persist,
            tc.tile_pool(name="xt", bufs=2) as xtp,
            tc.tile_pool(name="work", bufs=3) as work,
            tc.tile_pool(name="dram", bufs=1, space="DRAM") as dram,
        ):
            # ---- small constants (loaded once) ----
            masku_sb = constp.tile([128, 128], BF16)
            ident_sb = constp.tile([128, 128], BF16)
            identr_sb = constp.tile([128, 128], r)
            ones_sb = constp.tile([128, 128], r)
            nc.sync.dma_start(masku_sb[:], masku)
            nc.sync.dma_start(ident_sb[:], ident)
            nc.sync.dma_start(identr_sb[:], identr.bitcast(r))
            nc.sync.dma_start(ones_sb[:], onesr.bitcast(r))
            cc_in = dram.tile([NBLK, 128, 512], F32)
            cc_out = dram.tile([NBLK, 128, 512], F32)
            xTr = xT.rearrange("(e p) s -> p e s", p=128).bitcast(r)

            for _rep in range(reps):
                _body(nc, tc, constp, persist, xtp, work, dram,
                      xTr, wq, wk, wv, wo, bo, out,
                      masku_sb, ident_sb, identr_sb, ones_sb, cc_in, cc_out,
                      with_collective)

    nc.compile()
    return nc


def _body(nc, tc, constp, persist, xtp, work, dram,
          xTr, wq, wk, wv, wo, bo, out,
          masku_sb, ident_sb, identr_sb, ones_sb, cc_in, cc_out,
          with_collective):
    r = F32R

    # ---- projection weights ----
    wq_sb = constp.tile([128, 8, CD], r, tag="wq", name="wq_sb")
    wk_sb = constp.tile([128, 8, CD], r, tag="wk", name="wk_sb")
    wv_sb = constp.tile([128, 8, CD], r, tag="wv", name="wv_sb")
    nc.sync.dma_start(wq_sb[:], wq.rearrange("(e p) c -> p e c", p=128).bitcast(r))

    # ---- persistent activations ----
    qt_sb = persist.tile([128, BS], r, tag="qt", name="qt_sb")
    kt_sb = persist.tile([128, BS], r, tag="kt", name="kt_sb")
    v_sb = persist.tile([128, 32, 130], r, tag="v", name="v_sb")
    nc.vector.tensor_copy(v_sb[:, :, 64], ones_sb[:, 0:32])
    nc.vector.tensor_copy(v_sb[:, :, 129], ones_sb[:, 0:32])

    P = {}  # current-phase psum pools: P["pp"], P["ps"], P["pz"]

    def v_transposes(sc, vt_t):
        for st in range(4):
            tt = 4 * sc + st
            v_ps = P["pp"].tile([128, 128], r, tag=P["pptag"], name=f"vtp{sc}{st}")
            nc.tensor.transpose(
                v_ps[:], vt_t[:, 128 * st:128 * st + 128], identr_sb[:],
            )
            nc.vector.tensor_copy(v_sb[:, tt, 0:64], v_ps[:, 0:64])
            nc.vector.tensor_copy(v_sb[:, tt, 65:129], v_ps[:, 64:128])

    def proj_parts(sc):
        """Yield fine-grained projection closures for one 512-token chunk."""
        sl = bass.ts(sc, 512)
        state = {}

        def load():
            xt_t = xtp.tile([128, 8, 512], r, tag="xt", name=f"xt{sc}")
            nc.sync.dma_start(xt_t[:, 0:4, :], xTr[:, 0:4, sl])
            nc.sync.dma_start(xt_t[:, 4:8, :], xTr[:, 4:8, sl])
            if sc == 0:
                # defer K/V weight loads so the first Q matmuls start sooner
                nc.sync.dma_start(
                    wk_sb[:], wk.rearrange("(e p) c -> p e c", p=128).bitcast(r))
                nc.sync.dma_start(
                    wv_sb[:], wv.rearrange("(e p) c -> p e c", p=128).bitcast(r))
            state["xt"] = xt_t
            state["vt"] = xtp.tile([128, 512], r, tag="vtc", name=f"vtc{sc}")

        def group(w_sb, o_ap_fn, name):
            def run():
                p_ps = P["pp"].tile([128, 512], F32, tag=P["pptag"], name=f"pp{sc}{name}")
                for e in range(8):
                    nc.tensor.matmul(
                        p_ps[:], w_sb[:, e, :], state["xt"][:, e, :],
                        start=(e == 0), stop=(e == 7),
                    )
                nc.vector.tensor_copy(o_ap_fn(), p_ps[:])
            return run

        yield load
        yield group(wq_sb, lambda: qt_sb[:, sl], "q")
        yield group(wk_sb, lambda: kt_sb[:, sl], "k")
        yield group(wv_sb, lambda: state["vt"][:], "v")
        yield lambda: v_transposes(sc, state["vt"][:])

    def proj_chunk(sc):
        for part in proj_parts(sc):
            part()

    def attn_chunk_beats(b, m, stream):
        """Yield one closure per beat; caller weaves streams together."""
        q0 = 2048 * b + 512 * m
        last_t = 4 * m + 3
        state = {}

        def beat(t):
            if t == 0:
                state["z"] = [
                    P["pz"].tile([65, 512], F32, tag=f"z{stream}{h}",
                                 name=f"z{b}{m}{h}", bufs=1)
                    for h in (0, 1)
                ]
            z_ps = state["z"]

            def av(ta, pt_sb):
                joa = max(0, 128 * (ta - 4 * m))
                for h in (0, 1):
                    nc.tensor.matmul(
                        z_ps[h][:, joa:512],
                        v_sb[:, 16 * b + ta, 65 * h:65 * h + 65],
                        pt_sb[:, 512 * h + joa:512 * h + 512],
                        start=(ta == 0), stop=(ta == last_t),
                    )

            k0 = 2048 * b + 128 * t
            jo = max(0, 128 * (t - 4 * m))
            pt_sb = work.tile([128, 1024], r, tag="pt", name=f"pt{b}{m}{t}", bufs=4)
            if P["fused"]:
                s_ps = P["ps"].tile([128, 1024], F32, tag="s", name=f"s{b}{m}{t}")
                s_slices = [s_ps[:, 512 * h + jo:512] if False else s_ps[:, 512 * h + jo:512 * h + 512] for h in (0, 1)]
            else:
                s_tiles = [
                    P["ps"].tile([128, 512], F32, tag=f"s{h}", name=f"s{b}{m}{t}{h}")
                    for h in (0, 1)
                ]
                s_slices = [s_tiles[h][:, jo:512] for h in (0, 1)]
            for h in (0, 1):
                hsl = slice(64 * h, 64 * h + 64)
                nc.tensor.matmul(
                    s_slices[h],
                    kt_sb[hsl, k0:k0 + 128],
                    qt_sb[hsl, q0 + jo:q0 + 512],
                    start=True, stop=(t < 4 * m),
                )
                if t >= 4 * m:
                    nc.tensor.matmul(
                        s_slices[h][:, 0:128],
                        masku_sb[:], ident_sb[:],
                        start=False, stop=True,
                    )
                if not P["fused"]:
                    nc.scalar.activation(
                        pt_sb[:, 512 * h + jo:512 * h + 512], s_slices[h],
                        EXP, scale=float(SCALE),
                    )
            if P["fused"]:
                nc.scalar.activation(
                    pt_sb[:].rearrange("p (h w) -> p h w", h=2)[:, :, jo:512],
                    s_ps[:].rearrange("p (h w) -> p h w", h=2)[:, :, jo:512],
                    EXP, scale=float(SCALE),
                )
            pend = state.pop("pend", None)
            if pend is not None:
                av(*pend)
            state["pend"] = (t, pt_sb)
            if t == last_t:
                av(*state.pop("pend"))
                _norm(b, m, z_ps)

        for t in range(last_t + 1):
            yield lambda t=t: beat(t)

    def _norm(b, m, z_ps):
        # normalize and stage for all-to-all; copy psum out (incl. denom row)
        # immediately to release the z banks, then finish from SBUF
        zcp = [work.tile([65, 512], F32, tag=f"zc{h}", name=f"zc{b}{m}{h}", bufs=2)
               for h in (0, 1)]
        for h in (0, 1):
            nc.vector.tensor_copy(zcp[h][:], z_ps[h][:])
        zt_sb = work.tile([128, 512], F32, tag="zt", name=f"zt{b}{m}", bufs=2)
        for h in (0, 1):
            recip = work.tile([65, 512], r, tag="rc", name=f"rc{b}{m}{h}", bufs=2)
            with nc.allow_low_precision(reason="f32r is bitwise f32 here"):
                nc.vector.reciprocal(recip[64:65, :], zcp[h][64:65, :].bitcast(r))
            r_dram = dram.tile([1, 512], F32, tag="rd", name=f"rd{b}{m}{h}", bufs=2)
            nc.sync.dma_start(r_dram[:], recip[64:65, :].bitcast(F32))
            bc_sb = work.tile([64, 512], F32, tag="bc", name=f"bcs{b}{m}{h}", bufs=2)
            nc.sync.dma_start(bc_sb[:], r_dram.broadcast_to([64, 512]))
            nc.vector.tensor_mul(
                zt_sb[64 * h:64 * h + 64, :], zcp[h][0:64, :], bc_sb[:]
            )
        j = 4 * b + m
        nc.sync.dma_start(cc_in[j], zt_sb[:])

    # ---- issue order ----
    wo_sb = persist.tile([128, 8, D], r, tag="wo", name="wo_sb")
    bo_sb = constp.tile([1, D], r, tag="bo", name="bo_sb")

    def weave(tasks_a, tasks_b, fillers):
        """Round-robin beats from attention streams, sprinkling filler
        closures (projection work) between rounds."""
        ia = iter(tasks_a)
        ib = iter(tasks_b)
        fi = iter(fillers)
        done_a = done_b = False
        while not (done_a and done_b):
            try:
                next(ia)()
            except StopIteration:
                done_a = True
            try:
                next(ib)()
            except StopIteration:
                done_b = True
            f = next(fi, None)
            if f is not None:
                f()
        for f in fi:
            f()

    def proj_fillers_a():
        for sc in range(1, 8):
            yield from proj_parts(sc)
        yield lambda: nc.sync.dma_start(
            wo_sb[:], wo.rearrange("(i p) e -> p i e", p=128).bitcast(r))
        yield lambda: nc.sync.dma_start(bo_sb[:], bo.bitcast(r))

    def proj_fillers_b():
        return iter(())

    # phase A: projections + batch-0 attention (single stream; z uses
    # alternating tag pairs so chunk boundaries overlap)
    with (
        tc.tile_pool(name="ppA", bufs=2, space="PSUM") as ppA,
        tc.tile_pool(name="psA", bufs=2, space="PSUM") as psA,
        tc.tile_pool(name="pzA", bufs=1, space="PSUM") as pzA,
    ):
        P["pp"] = ppA
        P["pptag"] = "p"
        P["ps"] = psA
        P["pz"] = pzA
        P["bc"] = ppA
        P["bctag"] = "p"
        P["fused"] = False
        proj_chunk(0)
        beats_b0 = (
            list(attn_chunk_beats(0, 0, "A")) + list(attn_chunk_beats(0, 1, "A"))
            + list(attn_chunk_beats(0, 2, "A")) + list(attn_chunk_beats(0, 3, "A"))
        )
        weave(beats_b0, [], list(proj_fillers_a()))

    # phase B: batch-1 attention, two balanced streams; Wo projection
    with (
        tc.tile_pool(name="psB", bufs=2, space="PSUM") as psB,
        tc.tile_pool(name="pzB", bufs=1, space="PSUM") as pzB,
    ):
        P["pp"] = psB
        P["pptag"] = "s"
        P["ps"] = psB
        P["pz"] = pzB
        P["bc"] = psB
        P["bctag"] = "s"
        P["fused"] = True
        weave(
            list(attn_chunk_beats(1, 0, "A")) + list(attn_chunk_beats(1, 3, "A")),
            list(attn_chunk_beats(1, 1, "B")) + list(attn_chunk_beats(1, 2, "B")),
            list(proj_fillers_b()),
        )

        # ---- exchange Z^T: core j receives all head-dims for seq block j ----
        if with_collective:
            nc.gpsimd.collective_compute(
                "AllToAll",
                mybir.AluOpType.bypass,
                replica_groups=[list(range(N_CORES))],
                ins=[cc_in.opt()],
                outs=[cc_out.opt()],
            )
        else:
            nc.sync.dma_start(cc_out[:], cc_in[:])

        # ---- output projection: O[s_blk, :] = Z^T.T @ Wo + bo ----
        zt2_sb = persist.tile([128, 8, 512], r, tag="zt2", name="zt2_sb")
        ccr = cc_out.rearrange("i p s -> p i s").bitcast(r)
        nc.sync.dma_start(zt2_sb[:, 0:4, :], ccr[:, 0:4, :])
        nc.sync.dma_start(zt2_sb[:, 4:8, :], ccr[:, 4:8, :])

        for st in range(4):
            o_sb = work.tile([128, 1024], F32, tag="o", name=f"os{st}", bufs=2)
            for e in range(2):
                o_ps = P["ps"].tile([128, 512], F32, tag="s", name=f"o{st}{e}")
                for i in range(8):
                    nc.tensor.matmul(
                        o_ps[:],
                        zt2_sb[:, i, bass.ts(st, 128)],
                        wo_sb[:, i, bass.ts(e, 512)],
                        start=(i == 0), stop=False,
                    )
                nc.tensor.matmul(
                    o_ps[:], ones_sb[0:1, 0:128], bo_sb[0:1, bass.ts(e, 512)],
                    start=False, stop=True,
                )
                nc.vector.tensor_copy(o_sb[:, bass.ts(e, 512)], o_ps[:])
            nc.sync.dma_start(out[bass.ts(st, 128), :], o_sb[:])


def _prep_inputs(inputs, Wq, Wk, Wv, Wo, bo):
    x = np.asarray(inputs, dtype=np.float32).reshape(BS, D)
    xT = np.ascontiguousarray(x.T)
    Wq = np.asarray(Wq, dtype=np.float32)
    Wk = np.asarray(Wk, dtype=np.float32)
    Wv = np.asarray(Wv, dtype=np.float32)
    Wo = np.ascontiguousarray(np.asarray(Wo, dtype=np.float32))
    bo = np.asarray(bo, dtype=np.float32).reshape(1, D)
    masku = np.triu(np.full((128, 128), -BIG, dtype=np.float32), k=1).astype(ml_dtypes.bfloat16)
    ident = np.eye(128, dtype=np.float32).astype(ml_dtypes.bfloat16)
    identr = np.eye(128, dtype=np.float32)
    onesr = np.ones((128, 128), dtype=np.float32)
    in_maps = []
    for c in range(N_CORES):
        csl = slice(CD * c, CD * (c + 1))
        in_maps.append({
            "xT": xT,
            "wq": np.ascontiguousarray(Wq[:, csl]),
            "wk": np.ascontiguousarray(Wk[:, csl]),
            "wv": np.ascontiguousarray(Wv[:, csl]),
            "wo": Wo,
            "bo": bo,
            "masku": masku,
            "ident": ident,
            "identr": identr,
            "onesr": onesr,
        })
    return in_maps


def kernel(inputs, Wq, Wk, Wv, Wo, bo):
    if "nc" not in _CACHE:
        _CACHE["nc"] = build_nc()
    nc = _CACHE["nc"]
    in_maps = _prep_inputs(inputs, Wq, Wk, Wv, Wo, bo)
    res = run_bass_kernel_spmd(nc, in_maps, core_ids=list(range(N_CORES)))
    slabs = [res.results[c]["out"] for c in range(N_CORES)]
    return np.concatenate(slabs, axis=0).reshape(B, S, D)
